# revision 1
# baseline (speedup 1.0000x reference)
"""Soft-DTW-style loss (soft-min of pairwise sq-distances) on Trainium2.

Data-parallel over batch B=8: one batch element per NeuronCore. Per core
the matmul produces u[i,j] = 127 + log2e*(2 p.t - t2 - p2) = 127 - d*log2e
directly in PSUM (p2/t2/127 folded in as extra contraction rows, K=10).
The exp + row-sum is split between two engines by column:
  Scalar ACT: exp(-d) = exp(ln2*u - 127*ln2) via LUT, accum_out row-sum.
  Vector DVE: 2-pass software exp2 —
    pass1 (tensor_scalar): i32 = int32(max(u,0) * 2^23); bitcast(i32)
      = 2^(k-127) * (1+f) with k = floor(u), f = frac(u)
    pass2 (custom DVE op): x * (sq((x|1.0f)*a + b) + 1), accum=ADD;
      (x|1.0f) == 1+f exactly; the quadratic times host-side scale CS
      approximates the 2^f/(1+f) correction (max rel err 3.4e-3,
      zero-mean; end-to-end loss err ~2e-4).
Host combines: S = S_sc + CS*S_vec; loss = mean(-log S) in fp64.

Self-contained: hardcodes shapes B=8, L=2048, F=8.
"""

import numpy as np
from contextlib import ExitStack

B, L, F = 8, 2048, 8
P = 128          # partition tile height (i rows per tile)
NT = L // P      # 16 i-tiles
KA = F + 2       # augmented contraction: 8 features + (t2,127) row + p2 row
JC = 512         # j-chunk (one PSUM bank of fp32)
NJ = L // JC     # 4 j-chunks, one PE row-group each
CSPLIT = 1408    # cols [0:CSPLIT) on Scalar LUT exp, rest on Vector

LOG2E = 1.4426950408889634
LN2 = 0.6931471805599453
# correction-quadratic constants from fit_dve_exp.py
DVE_A = 0.49821132907389054
DVE_B = -0.7399745645614071
DVE_CS = 0.9415242422892705

_cache = {}


def _register_exp_op():
    """Register the custom DVE op EXP_FIX_ANT at runtime (monkeypatch)."""
    import concourse.dve_ops as dve_ops
    from concourse.dve_ops import DveOp, OPS, CUSTOM_DVE_SPECS, _SUB_OPCODE_FOR_NAME
    from concourse.dve_spec import Spec, Src0, C0, C1, C2, One, Bin, sq, lower
    from concourse.dve_uop import AluOp, DveOpSpec

    NAME = "EXP_FIX_ANT"
    if NAME in _SUB_OPCODE_FOR_NAME:
        return next(op for op in OPS if op.name == NAME)

    from operator import add

    h = Bin(AluOp.BITWISE_OR, Src0, C0)          # forces exponent to 127: 1+f
    body = (sq(h * C1 + C2) + One) * Src0

    def _ref(in0, in1, c0, c1, c2):
        x = np.ascontiguousarray(in0, np.float32)
        hh = (x.view(np.int32) | np.int32(0x3F800000)).view(np.float32)
        q = hh * np.float32(c1) + np.float32(c2)
        out = (q * q + np.float32(1.0)) * x
        acc = out.reshape(out.shape[0], -1).sum(axis=-1, keepdims=True)
        return out, acc

    spec = Spec(body=body, accum=add, reference=_ref)

    # find a free opcode row not already taken
    used_rows = set(_SUB_OPCODE_FOR_NAME.values())
    row = next(r for r in range(1, 0x20) if r not in used_rows)
    _SUB_OPCODE_FOR_NAME[NAME] = row

    shas = {}
    for ver in ("v3", "v4"):
        try:
            s = DveOpSpec(name=NAME, opcode=row, uops=lower(spec, ver=ver),
                          rd1_en=False)
            shas[ver] = s.sha(ver)
        except Exception:
            if ver == "v3":
                raise
    op = DveOp(NAME, spec, subdim=False, uops_sha=shas)
    OPS.append(op)
    CUSTOM_DVE_SPECS[NAME] = spec
    return op


def _build_nc():
    import concourse.tile as tile
    from concourse import bacc, mybir

    exp_op = _register_exp_op()

    dtf = mybir.dt.float32
    dtr = mybir.dt.float32r
    dtb = mybir.dt.bfloat16
    dti = mybir.dt.int32
    nc = bacc.Bacc("TRN2", target_bir_lowering=False, debug=False, num_devices=B)
    pa = nc.dram_tensor("pa", [KA, L], dtr, kind="ExternalInput").ap()
    ta = nc.dram_tensor("ta", [KA, L], dtr, kind="ExternalInput").ap()
    s_out = nc.dram_tensor("s_out", [P, 2 * NT], dtf, kind="ExternalOutput").ap()

    CV = L - CSPLIT  # vector-engine column count per tile

    with tile.TileContext(nc) as tc, ExitStack() as ctx:
        sb = ctx.enter_context(tc.tile_pool(name="sb", bufs=1))

        # Operands replicated at partition offsets 0/32/64/96: j-chunk q's
        # matmul contracts in PE row-group q so its LDWEIGHTS overlaps the
        # previous chunk's matmul. targ replica q only needs its own chunk.
        predAT4 = sb.tile([128, L], dtr)
        targAT4 = sb.tile([128, JC], dtr)
        S_all = sb.tile([P, 2 * NT], dtf)  # [:, :NT] scalar, [:, NT:] vector
        bias_c = sb.tile([P, 1], dtf)  # ACT bias: -127*ln2
        warm_s = sb.tile([32, 64], dtf)  # zeroed operand for PE warm-up
        nc.gpsimd.memset(bias_c[:], -127.0 * LN2)
        nc.gpsimd.memset(warm_s[:], 0.0)

        # 8 input descriptors spread over all three DGE queues (descriptor
        # issue costs ~0.7-1us each on a sequencer, so parallelize + order
        # MM(0,*)-gating transfers first). SBUF dst must be a plain 2D
        # partition slice (partition-dim rearrange emits flat-stride APs).
        nc.sync.dma_start(targAT4[0:KA, :], ta[:, 0:JC])
        nc.sync.dma_start(targAT4[32 : 32 + KA, :], ta[:, JC : 2 * JC])
        nc.sync.dma_start(predAT4[0:KA, :], pa)
        nc.sync.dma_start(predAT4[32 : 32 + KA, :], pa)
        nc.gpsimd.dma_start(targAT4[64 : 64 + KA, :], ta[:, 2 * JC : 3 * JC])
        nc.gpsimd.dma_start(targAT4[96 : 96 + KA, :], ta[:, 3 * JC : 4 * JC])
        nc.gpsimd.dma_start(predAT4[64 : 64 + KA, :], pa)
        nc.gpsimd.dma_start(predAT4[96 : 96 + KA, :], pa)

        scratch = ctx.enter_context(tc.tile_pool(name="scr", bufs=2))

        # Pre-load the exp ACT table set (~1.3us) while DMAs are in flight:
        # walrus inserts the PSEUDO_LOAD before this dependency-free dummy.
        dummy = scratch.tile([P, 1], dtf, tag="dummy")
        nc.scalar.activation(
            dummy[:], bias_c[:, 0:1], mybir.ActivationFunctionType.Exp,
            bias=bias_c[:, 0:1], scale=1.0,
        )

        # Warm the PE clock gate during the input-DMA wait: ~3.4us of
        # sustained activity lifts the HAM throttle from 1.2 to 2.4 GHz
        # before the first real matmul issues. Each iter costs ~214ns of PE
        # time (LDW+MM at cold clock); 16 of them fill the HAM window and
        # finish right as the input DMAs land (~10.5us) without blocking
        # the real matmuls behind them in the queue.
        with tc.tile_pool(name="warm", bufs=1, space="PSUM") as wp:
            wpt = wp.tile([64, 64], dtf, tag="w")
            for _ in range(16):
                nc.tensor.matmul(wpt[:], warm_s[:, :64], warm_s[:, :64],
                                 start=True, stop=True)
        with tc.tile_pool(name="pm", bufs=2, space="PSUM") as pm:
            for t in range(NT):
                ptp = pm.tile([P, L], dtf, tag="ptp")  # 4 PSUM banks
                for q in range(NJ):
                    nc.tensor.matmul(
                        ptp[:, q * JC : (q + 1) * JC],
                        predAT4[32 * q : 32 * q + KA, t * P : (t + 1) * P],
                        targAT4[32 * q : 32 * q + KA, :],
                        start=True,
                        stop=True,
                        # explicit: base_partition() auto-derive rejects 96
                        tile_position=(32 * q, 0),
                    )
                # Scalar: LUT exp over cols [0:CSPLIT)
                eT = scratch.tile([P, CSPLIT], dtb, tag="eT")
                nc.scalar.activation(
                    eT[:],
                    ptp[:, :CSPLIT],
                    mybir.ActivationFunctionType.Exp,
                    bias=bias_c[:, 0:1],
                    scale=LN2,
                    accum_out=S_all[:, t : t + 1],
                )
                # Vector pass1: int32(max(u,0) * 2^23)
                xI = scratch.tile([P, CV], dtf, tag="xI")
                nc.vector.tensor_scalar(
                    xI.bitcast(dti)[:],
                    ptp[:, CSPLIT:],
                    0.0,
                    float(2.0**23),
                    mybir.AluOpType.max,
                    mybir.AluOpType.mult,
                )
                # Vector pass2: correction + row-sum accumulate
                eV = scratch.tile([P, CV], dtb, tag="eV")
                nc.vector._custom_dve(
                    exp_op,
                    out=eV[:],
                    in0=xI[:],
                    s0=1.0,
                    s1=DVE_A,
                    imm2=DVE_B,
                    accum_out=S_all[:, NT + t : NT + t + 1],
                )

        # Each engine ships its own half right after its last accum write
        # (program order on that engine; no cross-engine semaphore hop).
        nc.scalar.dma_start(s_out[:, :NT], S_all[:, :NT])
        nc.sync.dma_start(s_out[:, NT:], S_all[:, NT:])

    nc.compile()
    return nc


def get_nc():
    if "nc" not in _cache:
        _cache["nc"] = _build_nc()
    return _cache["nc"]


def host_prep(pred_b: np.ndarray, target_b: np.ndarray) -> dict:
    """Pack one batch element into the device input layout."""
    pred_b = np.ascontiguousarray(pred_b, dtype=np.float32)
    target_b = np.ascontiguousarray(target_b, dtype=np.float32)

    p2 = np.sum(pred_b.astype(np.float64) * pred_b, axis=1)
    t2 = np.sum(target_b.astype(np.float64) * target_b, axis=1)
    pa = np.empty((KA, L), np.float32)
    pa[:F] = pred_b.T
    pa[F] = 1.0
    pa[F + 1] = LOG2E * p2
    ta = np.empty((KA, L), np.float32)
    ta[:F] = (2.0 * LOG2E) * target_b.T
    ta[F] = 127.0 - LOG2E * t2
    ta[F + 1] = -1.0
    return {
        "pa": np.ascontiguousarray(pa),
        "ta": np.ascontiguousarray(ta),
    }


def combine_s(s_out_b: np.ndarray) -> np.ndarray:
    """(128, 32) device output -> (128, 16) total row sums (fp64)."""
    s = s_out_b.astype(np.float64)
    return s[:, :NT] + DVE_CS * s[:, NT:]


def reduce_host(s_stack: np.ndarray) -> np.ndarray:
    """(B, 128, 32) raw outputs -> scalar mean(-log S), fp64 accumulate."""
    S = np.stack([combine_s(s_stack[b]) for b in range(B)])
    loss = -np.log(S)
    return np.asarray(loss.mean(), dtype=np.float32)


def run_on_hw(pred: np.ndarray, target: np.ndarray, trace: bool = False):
    from concourse import bass_utils

    nc = get_nc()
    in_maps = [host_prep(pred[b], target[b]) for b in range(B)]
    res = bass_utils.run_bass_kernel_spmd(
        nc, in_maps, core_ids=list(range(B)), trace=trace
    )
    s_stack = np.stack([r["s_out"] for r in res.results])  # (B, 128, 32)
    return reduce_host(s_stack), res


def kernel(pred: np.ndarray, target: np.ndarray) -> np.ndarray:
    pred = np.asarray(pred, dtype=np.float32)
    target = np.asarray(target, dtype=np.float32)
    assert pred.shape == (B, L, F) and target.shape == (B, L, F)
    loss, _ = run_on_hw(pred, target)
    return loss

